# revision 13
# baseline (speedup 1.0000x reference)
"""Trainium2 Bass kernel for nn_EquivariantProductBasisWithSelfMagmomBlock.

Data-parallel over nodes: 8 NeuronCores x 8192 nodes each.

Channel-major design: per 512-node supertile, PE transposes the node-major
inputs into channel-major [c, n] tiles; elementwise math runs mostly on fp16
[128, 512] tiles; matmuls run fp16 with fp32 PSUM accumulation.

v2 changes vs baseline:
 - attrs/inv/mag concatenated into one [128, 30] tile -> 4 input transposes
   per supertile instead of 24.
 - x1 components transpose into one 3-bank PSUM tile; a single Act copy
   moves all three planes to fp16 SBUF.
 - Act Silu directly from PSUM (replaces sigmoid + DVE mul per MLP layer).
 - wz chain restructured: DVE writes x0*wz products straight into PSUM and
   the companion wz term accumulates on top via a start=False matmul.
 - several SBUF-only fp16 adds/muls offloaded to the idle GpSimd engine.

PSUM budget (8 banks): x1p 3 + zs ring 2 + zb 1 + out ring 2.

Node map inside a core: local node n = s*512 + q*128 + p.
"""

import sys

sys.path.insert(0, "/opt/trn_rl_repo")

from contextlib import ExitStack

import numpy as np

import concourse.bass as bass
import concourse.tile as tile
from concourse import bacc, mybir
from concourse.bass_utils import run_bass_kernel_spmd
from concourse.masks import make_identity

FP32 = mybir.dt.float32
F32R = mybir.dt.float32r
FP16 = mybir.dt.float16
AF = mybir.ActivationFunctionType
OP = mybir.AluOpType

N = 65536
C = 128
E = 10
INV = 16
N_CORES = 8
N_CORE = N // N_CORES  # 8192
P = 128
G = 512  # nodes per supertile
CMB = 80  # padded: attrs@0:10, mag@32:36, inv@64:80 (matmul base-partition rule)

SCL = 16.0  # fp16 range guard: A-tiles carry 1/SCL, W_l* weights carry SCL


def bcast3(ap_2d):
    """[p, n] AP -> [p, 3, n] stride-0 broadcast AP on the middle dim."""
    return bass.AP(
        tensor=ap_2d.tensor, offset=ap_2d.offset,
        ap=[ap_2d.ap[0], [0, 3], ap_2d.ap[1]],
    )


def build_program(n_tiles):
    """Build the per-core SPMD program. n_tiles tiles of 128 nodes each."""
    nc = bacc.Bacc(
        "TRN2", target_bir_lowering=False, debug=False, num_devices=N_CORES
    )
    n_nodes = n_tiles * P
    assert n_tiles % 4 == 0
    n_st = n_tiles // 4

    def din(name, shape):
        return nc.dram_tensor(name, list(shape), FP32, kind="ExternalInput").ap()

    nf_d = din("node_feats", (n_nodes, 4 * C))
    sc_d = din("sc", (n_nodes, 4 * C))
    attrs_d = din("node_attrs", (n_nodes, E))
    inv_d = din("magmom_node_inv_feats", (n_nodes, INV))
    mag_d = din("magmom_node_attrs", (n_nodes, 4))
    wsc0_d = din("w_sc0", (E, 5 * C))
    wsc1_d = din("w_sc1", (E, 4 * C))
    w1_d = din("w_mlp1", (INV, 64))
    w2_d = din("w_mlp2", (64, 64))
    w3_d = din("w_mlp3", (64, 64))
    w4_d = din("w_mlp4", (64, 4 * C))
    wl0_d = din("W_l0", (2 * C, C))
    wl1_d = din("W_l1", (2 * C, C))
    wo0_d = din("Wo0", (C, C))
    wo1_d = din("Wo1", (C, C))
    out_d = nc.dram_tensor("out", [n_nodes, 4 * C], FP32, kind="ExternalOutput").ap()
    scr_d = nc.dram_tensor("warmup_scratch", [G, 4 * C], FP32, kind="Internal").ap()
    scr_r = scr_d.rearrange("(q p) x -> p q x", p=P, q=4)

    # node n = s*512 + q*128 + p
    nf_r = nf_d.rearrange("(s q p) x -> p s q x", p=P, q=4)
    sc_r = sc_d.rearrange("(s q p) x -> p s q x", p=P, q=4)
    out_r = out_d.rearrange("(s q p) x -> p s q x", p=P, q=4)
    attrs_r = attrs_d.rearrange("(s q p) x -> p s q x", p=P, q=4)
    inv_r = inv_d.rearrange("(s q p) x -> p s q x", p=P, q=4)
    mag_r = mag_d.rearrange("(s q p) x -> p s q x", p=P, q=4)

    with tile.TileContext(nc) as tc, ExitStack() as ctx:
        singles = ctx.enter_context(tc.tile_pool(name="singles", bufs=1))
        nat = ctx.enter_context(tc.tile_pool(name="nat", bufs=2))
        ew = ctx.enter_context(tc.tile_pool(name="ew", bufs=2))
        # PSUM pools (8 banks): tp 2 + wz 2 + zb 1 + misc 1 + out 2.
        # Per-stage pools decouple supertiles: s+1's transposes don't wait
        # for s's late-stage ring drain.
        tp_pool = ctx.enter_context(tc.tile_pool(name="tp", bufs=2, space="PSUM"))
        wzp_pool = ctx.enter_context(tc.tile_pool(name="wzp", bufs=2, space="PSUM"))
        acc_pool = ctx.enter_context(tc.tile_pool(name="accp", bufs=1, space="PSUM"))
        misc_pool = ctx.enter_context(tc.tile_pool(name="misc", bufs=1, space="PSUM"))
        out_pool = ctx.enter_context(tc.tile_pool(name="outp", bufs=2, space="PSUM"))

        # ---------------- preloads ----------------
        # identity is produced by gpsimd (Q7); launder it through a DVE copy
        # so PE never consumes a Q7-written tensor.
        ident_g = singles.tile([P, P], FP32)
        make_identity(nc, ident_g[:])
        ident = singles.tile([P, P], F32R)
        identh = singles.tile([P, P], FP16)

        # combined attrs|mag|inv per-node table, fp32; slice bases chosen so
        # each transposed block lands at a legal matmul base partition.
        cmb_all = singles.tile([P, n_st, 4, CMB], F32R)
        nc.sync.dma_start(out=cmb_all[:, :, :, 0:E], in_=attrs_r.bitcast(F32R))
        nc.sync.dma_start(out=cmb_all[:, :, :, 32:36], in_=mag_r.bitcast(F32R))
        nc.sync.dma_start(out=cmb_all[:, :, :, 64:64 + INV], in_=inv_r.bitcast(F32R))

        wscf = singles.tile([E, 9 * C], FP32)
        nc.sync.dma_start(out=wscf[:, 0:5 * C], in_=wsc0_d)
        nc.sync.dma_start(out=wscf[:, 5 * C:9 * C], in_=wsc1_d)
        wsc_h = singles.tile([E, 9 * C], FP16)
        nc.vector.tensor_copy(wsc_h[:], wscf[:])

        w1f = singles.tile([INV, 64], FP32)
        nc.sync.dma_start(out=w1f[:], in_=w1_d)
        w2f = singles.tile([64, 64], FP32)
        nc.sync.dma_start(out=w2f[:], in_=w2_d)
        w3f = singles.tile([64, 64], FP32)
        nc.sync.dma_start(out=w3f[:], in_=w3_d)
        w4f = singles.tile([64, 4 * C], FP32)
        nc.sync.dma_start(out=w4f[:], in_=w4_d)
        w2h = singles.tile([64, 64], FP16)
        nc.vector.tensor_copy(w2h[:], w2f[:])
        w3h = singles.tile([64, 64], FP16)
        nc.vector.tensor_copy(w3h[:], w3f[:])
        w4h = singles.tile([64, 4 * C], FP16)
        nc.vector.tensor_copy(w4h[:], w4f[:])
        # laundering copies sit late in the in-order DVE queue
        nc.vector.tensor_copy(ident[:], ident_g[:])
        nc.vector.tensor_copy(identh[:], ident_g[:])

        # output weights: 0=WA0*S 1=WB0*S 2=WA1*S 3=WB1*S 4=Wo0 5=Wo1
        Wf = singles.tile([P, 6, C], FP32)
        nc.sync.dma_start(out=Wf[:, 0, :], in_=wl0_d[0:128, :])
        nc.sync.dma_start(out=Wf[:, 1, :], in_=wl0_d[128:256, :])
        nc.sync.dma_start(out=Wf[:, 2, :], in_=wl1_d[0:128, :])
        nc.sync.dma_start(out=Wf[:, 3, :], in_=wl1_d[128:256, :])
        nc.sync.dma_start(out=Wf[:, 4, :], in_=wo0_d)
        nc.sync.dma_start(out=Wf[:, 5, :], in_=wo1_d)
        Wh = singles.tile([P, 6, C], FP16)
        nc.scalar.activation(Wh[:, 0:4, :], Wf[:, 0:4, :], AF.Copy, scale=SCL)
        nc.scalar.copy(Wh[:, 4:6, :], Wf[:, 4:6, :])

        # broadcast stationaries at base 32 (match magh rows): sel[k] picks
        # mag row 32+k and replicates it over all output partitions.
        sel32 = singles.tile([36, 4, P], FP16)
        ones36 = singles.tile([36, P], FP16)
        nc.vector.memset(ones36[:], 1.0 / SCL)
        # plane m selects mag row 32+m: sel[32+k, m, :] = (1/SCL)*delta(k==m),
        # built as ones * per-partition column e_m taken from the identity.
        for m in range(4):
            nc.vector.tensor_scalar_mul(
                sel32[32:36, m, :], ones36[32:36, :], ident_g[32:36, 32 + m:33 + m]
            )
        # MLP layer-1 stationary replica at base 64 (matches iT rows)
        w1h_rep = singles.tile([64 + INV, 64], FP16)
        nc.vector.tensor_copy(w1h_rep[64:64 + INV, :], w1f[:])

        def emit(s_, warmup=False):
            # ---------------- supertile loads ----------------
            nf_st = nat.tile([P, 16 * C], F32R, tag="nf")
            nc.sync.dma_start(
                out=nf_st[:].rearrange("p (q x) -> p q x", q=4),
                in_=nf_r[:, s_].bitcast(F32R),
            )
            sc_st = nat.tile([P, 16 * C], FP32, tag="sc")
            nc.sync.dma_start(
                out=sc_st[:].rearrange("p (q x) -> p q x", q=4), in_=sc_r[:, s_]
            )
            out_st = nat.tile([P, 16 * C], FP32, tag="out")

            nfv = nf_st[:].rearrange("p (q c j) -> p q c j", q=4, j=4)

            zs_n = [0]

            def ptile(pool, tag):
                zs_n[0] += 1
                return pool.tile([P, G], FP32, tag=tag, name=f"zs{zs_n[0]}")

            # ------- combined attrs|inv|mag transpose: 4 PE ops -------
            cmbp = ptile(tp_pool, "tp")
            for q in range(4):
                nc.tensor.matmul(
                    cmbp[0:CMB, q * P:(q + 1) * P],
                    cmb_all[:, s_, q, :], ident[:],
                )
            cmbh = ew.tile([CMB, G], FP16, tag="cmbh")
            nc.vector.tensor_copy(cmbh[:], cmbp[0:CMB, :])
            aT = cmbh[0:E, :]
            magh = cmbh[32:36, :]  # rows: a0, a1x, a1y, a1z (base 32)
            # (consumed as matmul moving at base 32 with sel32 stationaries)
            iT = cmbh[64:64 + INV, :]  # base 64

            # ------- x transposes -> PSUM; copies to fp16 SBUF -------
            x0p = ptile(tp_pool, "tp")
            for q in range(4):
                nc.tensor.matmul(
                    x0p[:, q * P:(q + 1) * P],
                    nfv[:, q, :, 0], ident[:],
                )
            x0h = ew.tile([P, G], FP16, tag="x0h")
            nc.vector.tensor_copy(x0h[:], x0p[:])
            xh = ew.tile([P, 3, G], FP16, tag="xh")
            for m in range(3):
                x1p = ptile(tp_pool, "tp")
                for q in range(4):
                    nc.tensor.matmul(
                        x1p[:, q * P:(q + 1) * P],
                        nfv[:, q, :, 1 + m], ident[:],
                    )
                nc.scalar.copy(xh[:, m, :], x1p[:])

            # ------- A broadcasts (PE ones-matmul, carries 1/SCL) -------
            A1 = ew.tile([P, 3, G], FP16, tag="A1")
            for m in range(3):
                bp = ptile(misc_pool, "mi")
                nc.tensor.matmul(bp[:], sel32[32:36, 1 + m, :], magh[:])
                nc.scalar.copy(A1[:, m, :], bp[:])
            bp = ptile(misc_pool, "mi")
            nc.tensor.matmul(bp[0:64, :], sel32[32:36, 0, 0:64], magh[:])
            A0h = ew.tile([64, G], FP16, tag="A0h")
            nc.scalar.copy(A0h[:], bp[0:64, :])

            # ------- magmom MLP (hoisted: only needs cmbh) -------
            h = iT
            hw_ = [w1h_rep[64:64 + INV, :], w2h[:], w3h[:]]
            for li in range(3):
                hp = ptile(misc_pool, "mi")
                nc.tensor.matmul(hp[0:64, :], hw_[li], h)
                hn = ew.tile([64, G], FP16, tag=f"h{li}")
                nc.scalar.activation(hn[:], hp[0:64, :], AF.Silu)
                h = hn[:]

            # ------- squares -------
            sq0 = ew.tile([P, G], FP16, tag="sq0")
            nc.vector.tensor_mul(sq0[:], x0h[:], x0h[:])
            sq1 = ew.tile([P, 3, G], FP16, tag="sq1", bufs=1)
            nc.scalar.activation(sq1[:], xh[:], AF.Square)
            n1h = ew.tile([P, G], FP16, tag="n1")
            nc.gpsimd.tensor_add(n1h[:], sq1[:, 0, :], sq1[:, 1, :])
            nc.gpsimd.tensor_add(n1h[:], n1h[:], sq1[:, 2, :])

            # ------- wz chain -------
            # A = wz0 + x0*wz1 + sq0*wz3 ; B = wz2 + x0*wz4
            # c1 = wz5 + x0*wz6 + sq0*wz7 + n1*wz8 ; y0 = x0*A + n1*B
            def wz_mm(k, out=None, start=True, stop=True):
                if out is None:
                    out = ptile(wzp_pool, "wz")
                nc.tensor.matmul(
                    out[:], wsc_h[:, k * P:(k + 1) * P], aT,
                    start=start, stop=stop, skip_group_check=True,
                )
                return out

            # A-block: AB(psum) = x0*wz1, += wz0 (PE), Av = AB + sq0*wz3
            wp = wz_mm(1)
            AB = acc_pool.tile([P, G], FP32, tag="zb")
            nc.vector.tensor_mul(AB[:], x0h[:], wp[:])
            wz_mm(0, out=AB, start=False, stop=True)
            wp = wz_mm(3)
            t3 = ew.tile([P, G], FP16, tag="t3", bufs=1)
            nc.vector.tensor_mul(t3[:], sq0[:], wp[:])
            Av = ew.tile([P, G], FP16, tag="Av", bufs=1)
            nc.vector.tensor_add(Av[:], t3[:], AB[:])
            ya = ew.tile([P, G], FP16, tag="ya", bufs=1)
            nc.vector.tensor_mul(ya[:], x0h[:], Av[:])

            # B-block: BB(psum) = x0*wz4, += wz2 (PE), yb = n1*BB
            wp = wz_mm(4)
            BB = acc_pool.tile([P, G], FP32, tag="zb")
            nc.vector.tensor_mul(BB[:], x0h[:], wp[:])
            wz_mm(2, out=BB, start=False, stop=True)
            yb = ew.tile([P, G], FP16, tag="yb", bufs=1)
            nc.vector.tensor_mul(yb[:], n1h[:], BB[:])
            y0 = ew.tile([P, G], FP16, tag="y0")
            nc.vector.tensor_add(y0[:], ya[:], yb[:])

            # c1-block: CB(psum) = x0*wz6, += wz5 (PE),
            # c1 = CB + sq0*wz7 (+ n1*wz8 on gpsimd)
            wp = wz_mm(6)
            CB = acc_pool.tile([P, G], FP32, tag="zb")
            nc.vector.tensor_mul(CB[:], x0h[:], wp[:])
            wz_mm(5, out=CB, start=False, stop=True)
            wp = wz_mm(7)
            t7 = ew.tile([P, G], FP16, tag="t7", bufs=1)
            nc.vector.tensor_mul(t7[:], sq0[:], wp[:])
            wp = wz_mm(8)
            t8 = ew.tile([P, G], FP16, tag="t8", bufs=1)
            nc.vector.tensor_mul(t8[:], n1h[:], wp[:])
            c1 = ew.tile([P, G], FP16, tag="c1")
            nc.vector.tensor_add(c1[:], t7[:], CB[:])
            nc.gpsimd.tensor_add(c1[:], c1[:], t8[:])

            # y1t = c1*x1 ; smul = y1t*A1 ; sv = sum_m smul
            y1t = ew.tile([P, 3, G], FP16, tag="y1t")
            nc.vector.tensor_mul(y1t[:], bcast3(c1[:]), xh[:])
            smul = ew.tile([P, 3, G], FP16, tag="smul", bufs=1)
            nc.vector.tensor_mul(smul[:], y1t[:], A1[:])
            sv = ew.tile([P, G], FP16, tag="sv")
            nc.gpsimd.tensor_add(sv[:], smul[:, 0, :], smul[:, 1, :])
            nc.gpsimd.tensor_add(sv[:], sv[:], smul[:, 2, :])

            # a0-scaled copy of h3 feeds the wa/wd matmuls (folds a0/SCL in)
            h3a = ew.tile([64, G], FP16, tag="h3a")
            nc.vector.tensor_mul(h3a[:], h, A0h[:])

            # tpw quarters: wa,wd use h3a (a0-scaled); wb,wc use h
            wp = ptile(misc_pool, "mi")
            nc.tensor.matmul(wp[:], w4h[:, 0:P], h3a[:])
            mid0a = ew.tile([P, G], FP16, tag="mid0a")
            nc.vector.tensor_mul(mid0a[:], y0[:], wp[:])
            wp = ptile(misc_pool, "mi")
            nc.tensor.matmul(wp[:], w4h[:, P:2 * P], h)
            g2 = ew.tile([P, G], FP16, tag="g2")
            nc.vector.tensor_mul(g2[:], sv[:], wp[:])
            wp = ptile(misc_pool, "mi")
            nc.tensor.matmul(wp[:], w4h[:, 2 * P:3 * P], h)
            wcy0 = ew.tile([P, G], FP16, tag="wcy0")
            nc.vector.tensor_mul(wcy0[:], y0[:], wp[:])
            wp = ptile(misc_pool, "mi")
            nc.tensor.matmul(wp[:], w4h[:, 3 * P:4 * P], h3a[:])
            rc2 = ew.tile([P, G], FP16, tag="rc2")
            nc.vector.tensor_mul(rc2[:], c1[:], wp[:])

            m1c = ew.tile([P, 3, G], FP16, tag="m1c", bufs=1)
            nc.vector.tensor_mul(m1c[:], bcast3(wcy0[:]), A1[:])
            hm = ew.tile([P, 3, G], FP16, tag="hm", bufs=1)
            nc.vector.tensor_mul(hm[:], bcast3(rc2[:]), xh[:])

            # ------- output linears: weight-stationary, channel-major PSUM,
            # then back-transpose via fp16 identity matmuls -------
            outv = out_st[:].rearrange("p (q f) -> p q f", q=4)
            scv = sc_st[:].rearrange("p (q f) -> p q f", q=4)

            def ocm_group(movings, whs):
                oc = ptile(wzp_pool, "wz")
                n_ = len(movings)
                for i_, (mv, wi) in enumerate(zip(movings, whs)):
                    nc.tensor.matmul(
                        oc[:], Wh[:, wi, :], mv,
                        start=(i_ == 0), stop=(i_ == n_ - 1),
                        skip_group_check=True,
                    )
                oh = ew.tile([P, G], FP16, tag="oh", name=f"oh{zs_n[0]}")
                nc.scalar.copy(oh[:], oc[:])
                return oh

            oh0 = ocm_group([mid0a[:], g2[:], y0[:]], [0, 1, 4])
            o0p = out_pool.tile([P, 4, P], FP32, tag="o")
            for q in range(4):
                nc.tensor.matmul(o0p[:, q, :], oh0[:, q * P:(q + 1) * P], identh[:])
            nc.vector.tensor_add(outv[:, :, 0:C], o0p[:], scv[:, :, 0:C])

            for m in range(3):
                oh1 = ocm_group(
                    [m1c[:, m, :], hm[:, m, :], y1t[:, m, :]], [2, 3, 5]
                )
                o1p = out_pool.tile([P, 4, P], FP32, tag="o")
                for q in range(4):
                    nc.tensor.matmul(o1p[:, q, :], oh1[:, q * P:(q + 1) * P], identh[:])
                ovm = outv[:, :, C:4 * C].rearrange("p q (c j) -> p q c j", j=3)[:, :, :, m]
                svm = scv[:, :, C:4 * C].rearrange("p q (c j) -> p q c j", j=3)[:, :, :, m]
                nc.vector.tensor_add(ovm, o1p[:], svm)

            tgt = scr_r if warmup else out_r[:, s_]
            nc.sync.dma_start(out=tgt, in_=out_st[:].rearrange("p (q x) -> p q x", q=4))

        # sacrificial first supertile absorbs cold-start races.
        emit(0, warmup=True)
        for s_ in range(n_st):
            emit(s_)

    nc.compile()
    return nc


_CACHE = {}


def _get_program(n_tiles):
    if n_tiles not in _CACHE:
        _CACHE[n_tiles] = build_program(n_tiles)
    return _CACHE[n_tiles]


def _in_map_for_core(inputs, c, n_core):
    lo, hi = c * n_core, (c + 1) * n_core
    return {
        "node_feats": np.ascontiguousarray(
            inputs["node_feats"][lo:hi].reshape(n_core, 4 * C)
        ),
        "sc": np.ascontiguousarray(inputs["sc"][lo:hi]),
        "node_attrs": np.ascontiguousarray(inputs["node_attrs"][lo:hi]),
        "magmom_node_inv_feats": np.ascontiguousarray(
            inputs["magmom_node_inv_feats"][lo:hi]
        ),
        "magmom_node_attrs": np.ascontiguousarray(inputs["magmom_node_attrs"][lo:hi]),
        "w_sc0": np.ascontiguousarray(inputs["w_sc0"].reshape(E, 5 * C)),
        "w_sc1": np.ascontiguousarray(inputs["w_sc1"].reshape(E, 4 * C)),
        "w_mlp1": np.asarray(inputs["w_mlp1"]),
        "w_mlp2": np.asarray(inputs["w_mlp2"]),
        "w_mlp3": np.asarray(inputs["w_mlp3"]),
        "w_mlp4": np.asarray(inputs["w_mlp4"]),
        "W_l0": np.asarray(inputs["W_l0"]),
        "W_l1": np.asarray(inputs["W_l1"]),
        "Wo0": np.asarray(inputs["Wo0"]),
        "Wo1": np.asarray(inputs["Wo1"]),
    }


def run_on_hw(inputs, trace=False, trace_cores=None):
    inputs = {k: np.asarray(v, dtype=np.float32) for k, v in inputs.items()}
    n_nodes = inputs["node_feats"].shape[0]
    n_core = n_nodes // N_CORES
    nc = _get_program(n_core // P)
    in_maps = [_in_map_for_core(inputs, c, n_core) for c in range(N_CORES)]
    res = run_bass_kernel_spmd(
        nc, in_maps, core_ids=list(range(N_CORES)), trace=trace,
        trace_cores=trace_cores,
    )
    out = np.concatenate([res.results[c]["out"] for c in range(N_CORES)], axis=0)
    return out.astype(np.float32), res


def kernel(**inputs) -> np.ndarray:
    import os, time

    os.environ.setdefault("NEURON_RT_RESET_CORES", "1")
    try:
        out, _ = run_on_hw(inputs, trace=False)
    except Exception:
        time.sleep(5)
        out, _ = run_on_hw(inputs, trace=False)
    return out


# revision 14
# speedup vs baseline: 1.4194x; 1.4194x over previous
"""Trainium2 Bass kernel for nn_EquivariantProductBasisWithSelfMagmomBlock.

Data-parallel over nodes: 8 NeuronCores x 8192 nodes each.

Channel-major design: per 512-node supertile, PE transposes the node-major
inputs into channel-major [c, n] tiles; elementwise math runs mostly on fp16
[128, 512] tiles; matmuls run fp16 with fp32 PSUM accumulation.

v2 changes vs baseline:
 - attrs/inv/mag concatenated into one [128, 30] tile -> 4 input transposes
   per supertile instead of 24.
 - x1 components transpose into one 3-bank PSUM tile; a single Act copy
   moves all three planes to fp16 SBUF.
 - Act Silu directly from PSUM (replaces sigmoid + DVE mul per MLP layer).
 - wz chain restructured: DVE writes x0*wz products straight into PSUM and
   the companion wz term accumulates on top via a start=False matmul.
 - several SBUF-only fp16 adds/muls offloaded to the idle GpSimd engine.

PSUM budget (8 banks): x1p 3 + zs ring 2 + zb 1 + out ring 2.

Node map inside a core: local node n = s*512 + q*128 + p.
"""

import sys

sys.path.insert(0, "/opt/trn_rl_repo")

from contextlib import ExitStack

import numpy as np

import concourse.bass as bass
import concourse.tile as tile
from concourse import bacc, mybir
from concourse.bass_utils import run_bass_kernel_spmd
from concourse.masks import make_identity

FP32 = mybir.dt.float32
F32R = mybir.dt.float32r
FP16 = mybir.dt.float16
AF = mybir.ActivationFunctionType
OP = mybir.AluOpType

N = 65536
C = 128
E = 10
INV = 16
N_CORES = 8
N_CORE = N // N_CORES  # 8192
P = 128
G = 512  # nodes per supertile
CMB = 80  # padded: attrs@0:10, mag@32:36, inv@64:80 (matmul base-partition rule)

SCL = 16.0  # fp16 range guard: A-tiles carry 1/SCL, W_l* weights carry SCL


def bcast3(ap_2d):
    """[p, n] AP -> [p, 3, n] stride-0 broadcast AP on the middle dim."""
    return bass.AP(
        tensor=ap_2d.tensor, offset=ap_2d.offset,
        ap=[ap_2d.ap[0], [0, 3], ap_2d.ap[1]],
    )


def build_program(n_tiles):
    """Build the per-core SPMD program. n_tiles tiles of 128 nodes each."""
    nc = bacc.Bacc(
        "TRN2", target_bir_lowering=False, debug=False, num_devices=N_CORES
    )
    n_nodes = n_tiles * P
    assert n_tiles % 4 == 0
    n_st = n_tiles // 4

    def din(name, shape):
        return nc.dram_tensor(name, list(shape), FP32, kind="ExternalInput").ap()

    nf_d = din("node_feats", (n_nodes, 4 * C))
    sc_d = din("sc", (n_nodes, 4 * C))
    attrs_d = din("node_attrs", (n_nodes, E))
    inv_d = din("magmom_node_inv_feats", (n_nodes, INV))
    mag_d = din("magmom_node_attrs", (n_nodes, 4))
    wsc0_d = din("w_sc0", (E, 5 * C))
    wsc1_d = din("w_sc1", (E, 4 * C))
    w1_d = din("w_mlp1", (INV, 64))
    w2_d = din("w_mlp2", (64, 64))
    w3_d = din("w_mlp3", (64, 64))
    w4_d = din("w_mlp4", (64, 4 * C))
    wl0_d = din("W_l0", (2 * C, C))
    wl1_d = din("W_l1", (2 * C, C))
    wo0_d = din("Wo0", (C, C))
    wo1_d = din("Wo1", (C, C))
    out_d = nc.dram_tensor("out", [n_nodes, 4 * C], FP32, kind="ExternalOutput").ap()
    scr_d = nc.dram_tensor("warmup_scratch", [G, 4 * C], FP32, kind="Internal").ap()
    scr_r = scr_d.rearrange("(q p) x -> p q x", p=P, q=4)

    # node n = s*512 + q*128 + p
    nf_r = nf_d.rearrange("(s q p) x -> p s q x", p=P, q=4)
    sc_r = sc_d.rearrange("(s q p) x -> p s q x", p=P, q=4)
    out_r = out_d.rearrange("(s q p) x -> p s q x", p=P, q=4)
    attrs_r = attrs_d.rearrange("(s q p) x -> p s q x", p=P, q=4)
    inv_r = inv_d.rearrange("(s q p) x -> p s q x", p=P, q=4)
    mag_r = mag_d.rearrange("(s q p) x -> p s q x", p=P, q=4)

    with tile.TileContext(nc) as tc, ExitStack() as ctx:
        singles = ctx.enter_context(tc.tile_pool(name="singles", bufs=1))
        nat = ctx.enter_context(tc.tile_pool(name="nat", bufs=2))
        ew = ctx.enter_context(tc.tile_pool(name="ew", bufs=2))
        # PSUM pools (8 banks): tp 2 + wz 2 + zb 1 + misc 1 + out 2.
        # Per-stage pools decouple supertiles: s+1's transposes don't wait
        # for s's late-stage ring drain.
        tp_pool = ctx.enter_context(tc.tile_pool(name="tp", bufs=2, space="PSUM"))
        wzp_pool = ctx.enter_context(tc.tile_pool(name="wzp", bufs=2, space="PSUM"))
        acc_pool = ctx.enter_context(tc.tile_pool(name="accp", bufs=1, space="PSUM"))
        misc_pool = ctx.enter_context(tc.tile_pool(name="misc", bufs=1, space="PSUM"))
        out_pool = ctx.enter_context(tc.tile_pool(name="outp", bufs=2, space="PSUM"))

        # ---------------- preloads ----------------
        # identity is produced by gpsimd (Q7); launder it through a DVE copy
        # so PE never consumes a Q7-written tensor.
        ident_g = singles.tile([P, P], FP32)
        make_identity(nc, ident_g[:])
        ident = singles.tile([P, P], F32R)
        identh = singles.tile([P, P], FP16)

        # combined attrs|mag|inv per-node table, fp32; slice bases chosen so
        # each transposed block lands at a legal matmul base partition.
        cmb_all = singles.tile([P, n_st, 4, CMB], F32R)
        nc.sync.dma_start(out=cmb_all[:, :, :, 0:E], in_=attrs_r.bitcast(F32R))
        nc.sync.dma_start(out=cmb_all[:, :, :, 32:36], in_=mag_r.bitcast(F32R))
        nc.sync.dma_start(out=cmb_all[:, :, :, 64:64 + INV], in_=inv_r.bitcast(F32R))

        wscf = singles.tile([E, 9 * C], FP32)
        nc.sync.dma_start(out=wscf[:, 0:5 * C], in_=wsc0_d)
        nc.sync.dma_start(out=wscf[:, 5 * C:9 * C], in_=wsc1_d)
        wsc_h = singles.tile([E, 9 * C], FP16)
        nc.vector.tensor_copy(wsc_h[:], wscf[:])

        w1f = singles.tile([INV, 64], FP32)
        nc.sync.dma_start(out=w1f[:], in_=w1_d)
        w2f = singles.tile([64, 64], FP32)
        nc.sync.dma_start(out=w2f[:], in_=w2_d)
        w3f = singles.tile([64, 64], FP32)
        nc.sync.dma_start(out=w3f[:], in_=w3_d)
        w4f = singles.tile([64, 4 * C], FP32)
        nc.sync.dma_start(out=w4f[:], in_=w4_d)
        w2h = singles.tile([64, 64], FP16)
        nc.vector.tensor_copy(w2h[:], w2f[:])
        w3h = singles.tile([64, 64], FP16)
        nc.vector.tensor_copy(w3h[:], w3f[:])
        w4h = singles.tile([64, 4 * C], FP16)
        nc.vector.tensor_copy(w4h[:], w4f[:])
        # laundering copies sit late in the in-order DVE queue
        nc.vector.tensor_copy(ident[:], ident_g[:])
        nc.vector.tensor_copy(identh[:], ident_g[:])

        # output weights: 0=WA0*S 1=WB0*S 2=WA1*S 3=WB1*S 4=Wo0 5=Wo1
        Wf = singles.tile([P, 6, C], FP32)
        nc.sync.dma_start(out=Wf[:, 0, :], in_=wl0_d[0:128, :])
        nc.sync.dma_start(out=Wf[:, 1, :], in_=wl0_d[128:256, :])
        nc.sync.dma_start(out=Wf[:, 2, :], in_=wl1_d[0:128, :])
        nc.sync.dma_start(out=Wf[:, 3, :], in_=wl1_d[128:256, :])
        nc.sync.dma_start(out=Wf[:, 4, :], in_=wo0_d)
        nc.sync.dma_start(out=Wf[:, 5, :], in_=wo1_d)
        Wh = singles.tile([P, 6, C], FP16)
        nc.scalar.activation(Wh[:, 0:4, :], Wf[:, 0:4, :], AF.Copy, scale=SCL)
        nc.scalar.copy(Wh[:, 4:6, :], Wf[:, 4:6, :])

        # broadcast stationaries at base 32 (match magh rows): sel[k] picks
        # mag row 32+k and replicates it over all output partitions.
        sel32 = singles.tile([36, 4, P], FP16)
        ones36 = singles.tile([36, P], FP16)
        nc.vector.memset(ones36[:], 1.0 / SCL)
        # plane m selects mag row 32+m: sel[32+k, m, :] = (1/SCL)*delta(k==m),
        # built as ones * per-partition column e_m taken from the identity.
        for m in range(4):
            nc.vector.tensor_scalar_mul(
                sel32[32:36, m, :], ones36[32:36, :], ident_g[32:36, 32 + m:33 + m]
            )
        # MLP layer-1 stationary replica at base 64 (matches iT rows)
        w1h_rep = singles.tile([64 + INV, 64], FP16)
        nc.vector.tensor_copy(w1h_rep[64:64 + INV, :], w1f[:])

        def emit(s_, warmup=False):
            # ---------------- supertile loads ----------------
            nf_st = nat.tile([P, 16 * C], F32R, tag="nf")
            nc.sync.dma_start(
                out=nf_st[:].rearrange("p (q x) -> p q x", q=4),
                in_=nf_r[:, s_].bitcast(F32R),
            )
            sc_st = nat.tile([P, 16 * C], FP32, tag="sc")
            nc.sync.dma_start(
                out=sc_st[:].rearrange("p (q x) -> p q x", q=4), in_=sc_r[:, s_]
            )
            out_st = nat.tile([P, 16 * C], FP32, tag="out")

            nfv = nf_st[:].rearrange("p (q c j) -> p q c j", q=4, j=4)

            zs_n = [0]

            def ptile(pool, tag):
                zs_n[0] += 1
                return pool.tile([P, G], FP32, tag=tag, name=f"zs{zs_n[0]}")

            # ------- combined attrs|inv|mag transpose: 4 PE ops -------
            cmbp = ptile(tp_pool, "tp")
            for q in range(4):
                nc.tensor.matmul(
                    cmbp[0:CMB, q * P:(q + 1) * P],
                    cmb_all[:, s_, q, :], ident[:],
                )
            cmbh = ew.tile([CMB, G], FP16, tag="cmbh")
            nc.scalar.copy(cmbh[:], cmbp[0:CMB, :])
            aT = cmbh[0:E, :]
            magh = cmbh[32:36, :]  # rows: a0, a1x, a1y, a1z (base 32)
            # (consumed as matmul moving at base 32 with sel32 stationaries)
            iT = cmbh[64:64 + INV, :]  # base 64

            # ------- x transposes -> PSUM; copies to fp16 SBUF -------
            x0p = ptile(tp_pool, "tp")
            for q in range(4):
                nc.tensor.matmul(
                    x0p[:, q * P:(q + 1) * P],
                    nfv[:, q, :, 0], ident[:],
                )
            x0h = ew.tile([P, G], FP16, tag="x0h")
            nc.scalar.copy(x0h[:], x0p[:])
            xh = ew.tile([P, 3, G], FP16, tag="xh")
            for m in range(3):
                x1p = ptile(tp_pool, "tp")
                for q in range(4):
                    nc.tensor.matmul(
                        x1p[:, q * P:(q + 1) * P],
                        nfv[:, q, :, 1 + m], ident[:],
                    )
                nc.scalar.copy(xh[:, m, :], x1p[:])

            # ------- A broadcasts (PE ones-matmul, carries 1/SCL) -------
            A1 = ew.tile([P, 3, G], FP16, tag="A1")
            for m in range(3):
                bp = ptile(misc_pool, "mi")
                nc.tensor.matmul(bp[:], sel32[32:36, 1 + m, :], magh[:])
                nc.scalar.copy(A1[:, m, :], bp[:])
            bp = ptile(misc_pool, "mi")
            nc.tensor.matmul(bp[0:64, :], sel32[32:36, 0, 0:64], magh[:])
            A0h = ew.tile([64, G], FP16, tag="A0h")
            nc.scalar.copy(A0h[:], bp[0:64, :])

            # ------- magmom MLP (hoisted: only needs cmbh) -------
            h = iT
            hw_ = [w1h_rep[64:64 + INV, :], w2h[:], w3h[:]]
            for li in range(3):
                hp = ptile(misc_pool, "mi")
                nc.tensor.matmul(hp[0:64, :], hw_[li], h)
                hn = ew.tile([64, G], FP16, tag=f"h{li}")
                nc.scalar.activation(hn[:], hp[0:64, :], AF.Silu)
                h = hn[:]

            # ------- squares -------
            sq0 = ew.tile([P, G], FP16, tag="sq0")
            nc.vector.tensor_mul(sq0[:], x0h[:], x0h[:])
            sq1 = ew.tile([P, 3, G], FP16, tag="sq1", bufs=1)
            nc.scalar.activation(sq1[:], xh[:], AF.Square)
            n1h = ew.tile([P, G], FP16, tag="n1")
            nc.gpsimd.tensor_add(n1h[:], sq1[:, 0, :], sq1[:, 1, :])
            nc.gpsimd.tensor_add(n1h[:], n1h[:], sq1[:, 2, :])

            # ------- wz chain -------
            # A = wz0 + x0*wz1 + sq0*wz3 ; B = wz2 + x0*wz4
            # c1 = wz5 + x0*wz6 + sq0*wz7 + n1*wz8 ; y0 = x0*A + n1*B
            def wz_mm(k, out=None, start=True, stop=True):
                if out is None:
                    out = ptile(wzp_pool, "wz")
                nc.tensor.matmul(
                    out[:], wsc_h[:, k * P:(k + 1) * P], aT,
                    start=start, stop=stop, skip_group_check=True,
                )
                return out

            # A-block: AB(psum) = x0*wz1, += wz0 (PE), Av = AB + sq0*wz3
            wp = wz_mm(1)
            AB = acc_pool.tile([P, G], FP32, tag="zb")
            nc.vector.tensor_mul(AB[:], x0h[:], wp[:])
            wz_mm(0, out=AB, start=False, stop=True)
            wp = wz_mm(3)
            t3 = ew.tile([P, G], FP16, tag="t3", bufs=1)
            nc.vector.tensor_mul(t3[:], sq0[:], wp[:])
            Av = ew.tile([P, G], FP16, tag="Av", bufs=1)
            nc.vector.tensor_add(Av[:], t3[:], AB[:])
            ya = ew.tile([P, G], FP16, tag="ya", bufs=1)
            nc.vector.tensor_mul(ya[:], x0h[:], Av[:])

            # B-block: BB(psum) = x0*wz4, += wz2 (PE), yb = n1*BB
            wp = wz_mm(4)
            BB = acc_pool.tile([P, G], FP32, tag="zb")
            nc.vector.tensor_mul(BB[:], x0h[:], wp[:])
            wz_mm(2, out=BB, start=False, stop=True)
            yb = ew.tile([P, G], FP16, tag="yb", bufs=1)
            nc.vector.tensor_mul(yb[:], n1h[:], BB[:])
            y0 = ew.tile([P, G], FP16, tag="y0")
            nc.vector.tensor_add(y0[:], ya[:], yb[:])

            # c1-block: CB(psum) = x0*wz6, += wz5 (PE),
            # c1 = CB + sq0*wz7 (+ n1*wz8 on gpsimd)
            wp = wz_mm(6)
            CB = acc_pool.tile([P, G], FP32, tag="zb")
            nc.vector.tensor_mul(CB[:], x0h[:], wp[:])
            wz_mm(5, out=CB, start=False, stop=True)
            wp = wz_mm(7)
            t7 = ew.tile([P, G], FP16, tag="t7", bufs=1)
            nc.vector.tensor_mul(t7[:], sq0[:], wp[:])
            wp = wz_mm(8)
            t8 = ew.tile([P, G], FP16, tag="t8", bufs=1)
            nc.vector.tensor_mul(t8[:], n1h[:], wp[:])
            c1 = ew.tile([P, G], FP16, tag="c1")
            nc.vector.tensor_add(c1[:], t7[:], CB[:])
            nc.gpsimd.tensor_add(c1[:], c1[:], t8[:])

            # y1t = c1*x1 ; smul = y1t*A1 ; sv = sum_m smul
            y1t = ew.tile([P, 3, G], FP16, tag="y1t")
            c1b2 = bass.AP(tensor=c1[:].tensor, offset=c1[:].offset,
                           ap=[c1[:].ap[0], [0, 2], c1[:].ap[1]])
            nc.vector.tensor_mul(y1t[:, 0:2, :], c1b2, xh[:, 0:2, :])
            nc.gpsimd.tensor_mul(y1t[:, 2, :], c1[:], xh[:, 2, :])
            smul = ew.tile([P, 3, G], FP16, tag="smul", bufs=1)
            nc.vector.tensor_mul(smul[:, 0:2, :], y1t[:, 0:2, :], A1[:, 0:2, :])
            nc.gpsimd.tensor_mul(smul[:, 2, :], y1t[:, 2, :], A1[:, 2, :])
            sv = ew.tile([P, G], FP16, tag="sv")
            nc.gpsimd.tensor_add(sv[:], smul[:, 0, :], smul[:, 1, :])
            nc.gpsimd.tensor_add(sv[:], sv[:], smul[:, 2, :])

            # a0-scaled copy of h3 feeds the wa/wd matmuls (folds a0/SCL in)
            h3a = ew.tile([64, G], FP16, tag="h3a")
            nc.vector.tensor_mul(h3a[:], h, A0h[:])

            # tpw quarters: wa,wd use h3a (a0-scaled); wb,wc use h
            wp = ptile(misc_pool, "mi")
            nc.tensor.matmul(wp[:], w4h[:, 0:P], h3a[:])
            mid0a = ew.tile([P, G], FP16, tag="mid0a")
            nc.vector.tensor_mul(mid0a[:], y0[:], wp[:])
            wp = ptile(misc_pool, "mi")
            nc.tensor.matmul(wp[:], w4h[:, P:2 * P], h)
            g2 = ew.tile([P, G], FP16, tag="g2")
            nc.vector.tensor_mul(g2[:], sv[:], wp[:])
            wp = ptile(misc_pool, "mi")
            nc.tensor.matmul(wp[:], w4h[:, 2 * P:3 * P], h)
            wcy0 = ew.tile([P, G], FP16, tag="wcy0")
            nc.vector.tensor_mul(wcy0[:], y0[:], wp[:])
            wp = ptile(misc_pool, "mi")
            nc.tensor.matmul(wp[:], w4h[:, 3 * P:4 * P], h3a[:])
            rc2 = ew.tile([P, G], FP16, tag="rc2")
            nc.vector.tensor_mul(rc2[:], c1[:], wp[:])

            m1c = ew.tile([P, 3, G], FP16, tag="m1c", bufs=1)
            nc.vector.tensor_mul(m1c[:], bcast3(wcy0[:]), A1[:])
            hm = ew.tile([P, 3, G], FP16, tag="hm", bufs=1)
            rcb2 = bass.AP(tensor=rc2[:].tensor, offset=rc2[:].offset,
                           ap=[rc2[:].ap[0], [0, 2], rc2[:].ap[1]])
            nc.vector.tensor_mul(hm[:, 0:2, :], rcb2, xh[:, 0:2, :])
            nc.gpsimd.tensor_mul(hm[:, 2, :], rc2[:], xh[:, 2, :])

            # ------- output linears: node-major PSUM via mid-stationary -------
            outv = out_st[:].rearrange("p (q f) -> p q f", q=4)
            scv = sc_st[:].rearrange("p (q f) -> p q f", q=4)

            o0p = out_pool.tile([P, 4, P], FP32, tag="o")
            for q in range(4):
                qs = slice(q * P, (q + 1) * P)
                nc.tensor.matmul(o0p[:, q, :], mid0a[:, qs], Wh[:, 0, :], start=True, stop=False)
                nc.tensor.matmul(o0p[:, q, :], g2[:, qs], Wh[:, 1, :], start=False, stop=False)
                nc.tensor.matmul(o0p[:, q, :], y0[:, qs], Wh[:, 4, :], start=False, stop=True)
            nc.vector.tensor_add(outv[:, :, 0:C], o0p[:], scv[:, :, 0:C])

            for m in range(3):
                o1p = out_pool.tile([P, 4, P], FP32, tag="o")
                for q in range(4):
                    qs = slice(q * P, (q + 1) * P)
                    nc.tensor.matmul(o1p[:, q, :], m1c[:, m, qs], Wh[:, 2, :], start=True, stop=False)
                    nc.tensor.matmul(o1p[:, q, :], hm[:, m, qs], Wh[:, 3, :], start=False, stop=False)
                    nc.tensor.matmul(o1p[:, q, :], y1t[:, m, qs], Wh[:, 5, :], start=False, stop=True)
                ovm = outv[:, :, C:4 * C].rearrange("p q (c j) -> p q c j", j=3)[:, :, :, m]
                svm = scv[:, :, C:4 * C].rearrange("p q (c j) -> p q c j", j=3)[:, :, :, m]
                nc.vector.tensor_add(ovm, o1p[:], svm)

            tgt = scr_r if warmup else out_r[:, s_]
            nc.sync.dma_start(out=tgt, in_=out_st[:].rearrange("p (q x) -> p q x", q=4))

        # sacrificial first supertile absorbs cold-start races.
        emit(0, warmup=True)
        for s_ in range(n_st):
            emit(s_)

    nc.compile()
    return nc


_CACHE = {}


def _get_program(n_tiles):
    if n_tiles not in _CACHE:
        _CACHE[n_tiles] = build_program(n_tiles)
    return _CACHE[n_tiles]


def _in_map_for_core(inputs, c, n_core):
    lo, hi = c * n_core, (c + 1) * n_core
    return {
        "node_feats": np.ascontiguousarray(
            inputs["node_feats"][lo:hi].reshape(n_core, 4 * C)
        ),
        "sc": np.ascontiguousarray(inputs["sc"][lo:hi]),
        "node_attrs": np.ascontiguousarray(inputs["node_attrs"][lo:hi]),
        "magmom_node_inv_feats": np.ascontiguousarray(
            inputs["magmom_node_inv_feats"][lo:hi]
        ),
        "magmom_node_attrs": np.ascontiguousarray(inputs["magmom_node_attrs"][lo:hi]),
        "w_sc0": np.ascontiguousarray(inputs["w_sc0"].reshape(E, 5 * C)),
        "w_sc1": np.ascontiguousarray(inputs["w_sc1"].reshape(E, 4 * C)),
        "w_mlp1": np.asarray(inputs["w_mlp1"]),
        "w_mlp2": np.asarray(inputs["w_mlp2"]),
        "w_mlp3": np.asarray(inputs["w_mlp3"]),
        "w_mlp4": np.asarray(inputs["w_mlp4"]),
        "W_l0": np.asarray(inputs["W_l0"]),
        "W_l1": np.asarray(inputs["W_l1"]),
        "Wo0": np.asarray(inputs["Wo0"]),
        "Wo1": np.asarray(inputs["Wo1"]),
    }


def run_on_hw(inputs, trace=False, trace_cores=None):
    inputs = {k: np.asarray(v, dtype=np.float32) for k, v in inputs.items()}
    n_nodes = inputs["node_feats"].shape[0]
    n_core = n_nodes // N_CORES
    nc = _get_program(n_core // P)
    in_maps = [_in_map_for_core(inputs, c, n_core) for c in range(N_CORES)]
    res = run_bass_kernel_spmd(
        nc, in_maps, core_ids=list(range(N_CORES)), trace=trace,
        trace_cores=trace_cores,
    )
    out = np.concatenate([res.results[c]["out"] for c in range(N_CORES)], axis=0)
    return out.astype(np.float32), res


def kernel(**inputs) -> np.ndarray:
    import os, time

    os.environ.setdefault("NEURON_RT_RESET_CORES", "1")
    try:
        out, _ = run_on_hw(inputs, trace=False)
    except Exception:
        time.sleep(5)
        out, _ = run_on_hw(inputs, trace=False)
    return out
